# revision 1
# baseline (speedup 1.0000x reference)
"""Block-sparse self-attention (inverted mask) for Trainium2, 8-core SPMD.

Problem: nn_BlockSparseSelfAttention — B=2, H=16, S=2048, D=64, BLOCK=64.
reference returns (out, attn, M) where the mask *fills* same-block and
head-column positions with -inf (softmax runs over the complement).

Sharding: the 32 (b,h) pairs are split 4-per-core across 8 NeuronCores.

Device kernel (per core, per (b,h)) works in the TRANSPOSED orientation
(t on partitions, s on the free dim):

    ST[t, s]  = (K @ Q^T) / sqrt(D)                    (PE; K^T is lhsT)
    E[t, s]   = exp(ST/sqrt(D) + bias_t)               (ACT; bias_t=-1e38 on rows
                                                        t%64==0 -> head-col mask)
    E[diag]   = 0                                      (memsets -> same-block mask)
    Z         = [V | 1]^T @ E                          (PE; row D of Z = softmax sums)
    rbc[t, s] = exp(-outer(ones, ln(Z[D, s])))         (PE outer + ACT; 1/sum bcast,
                                                        division-free)
    A[t, s]   = E * rbc                                (DVE/GPSIMD)
    outT[d,s] = Z[d, s] * rbc[d, s]                    (normalized (attn @ V)^T)

attn^T and out^T are DMA'd out; the host transposes back during unshard.
No max-subtraction: inputs are N(0,1) so scores/sqrt(D) ~ N(0,1); exp is
safely within fp32 range and softmax is shift-invariant.
"""

from contextlib import ExitStack

import numpy as np

import concourse.bass as bass  # noqa: F401  (env-provided)
import concourse.tile as tile
from concourse import bacc, mybir
from concourse.bass_utils import run_bass_kernel_spmd

F32 = mybir.dt.float32
P = 128          # partitions / t-chunk size
BLOCK = 64       # mask block size
NEG = -1.0e38

B, H, S, D = 2, 16, 2048, 64
N_CORES = 8
BH = B * H
BH_PER_CORE = BH // N_CORES
S_TILE = 1024


def build_nc(n_bh=BH_PER_CORE, s=S, d=D, s_tile=S_TILE, gp_every=4, debug=False):
    """Build the per-core Bass module. The same program runs on every core."""
    assert s % P == 0 and s % s_tile == 0
    n_chunk = s // P          # number of 128-row t chunks
    n_half = s // s_tile      # number of s column blocks
    w = min(512, s_tile)      # matmul moving width
    n_w = s_tile // w
    EXP = mybir.ActivationFunctionType.Exp
    LOG = mybir.ActivationFunctionType.Ln

    nc = bacc.Bacc("TRN2", target_bir_lowering=False, debug=debug)
    QT = nc.dram_tensor("qt", [n_bh, d, s], F32, kind="ExternalInput").ap()
    KT = nc.dram_tensor("kt", [n_bh, d, s], F32, kind="ExternalInput").ap()
    VA = nc.dram_tensor("va", [n_bh, s, d + 1], F32, kind="ExternalInput").ap()
    BI = nc.dram_tensor("bias", [P, 1], F32, kind="ExternalInput").ap()
    AT = nc.dram_tensor("attnT", [n_bh, s, s], F32, kind="ExternalOutput").ap()
    OT = nc.dram_tensor("outT", [n_bh, d, s], F32, kind="ExternalOutput").ap()

    scale = 1.0 / float(d) ** 0.5

    with tile.TileContext(nc) as tc:
        with ExitStack() as ctx:
            const = ctx.enter_context(tc.tile_pool(name="const", bufs=1))
            io_qk = ctx.enter_context(tc.tile_pool(name="io_qk", bufs=2))
            io_va = ctx.enter_context(tc.tile_pool(name="io_va", bufs=1))
            ebig_pool = ctx.enter_context(tc.tile_pool(name="ebig", bufs=2))
            zs_pool = ctx.enter_context(tc.tile_pool(name="zsb", bufs=2))
            sm1 = ctx.enter_context(tc.tile_pool(name="sm1", bufs=1))
            ot_pool = ctx.enter_context(tc.tile_pool(name="ot", bufs=2))
            pst = ctx.enter_context(tc.tile_pool(name="pst", bufs=2, space="PSUM"))
            pz = ctx.enter_context(tc.tile_pool(name="pz", bufs=1, space="PSUM"))
            pr = ctx.enter_context(tc.tile_pool(name="pr", bufs=1, space="PSUM"))

            bias_sb = const.tile([P, 1], F32)
            nc.scalar.dma_start(bias_sb[:], BI[:])
            ones_sb = const.tile([1, P], F32)
            nc.vector.memset(ones_sb[:], 1.0)

            for ib in range(n_bh):
                qt_sb = io_qk.tile([d, s], F32, tag="qt")
                kt_sb = io_qk.tile([d, s], F32, tag="kt")
                va_sb = io_va.tile([P, n_chunk, d + 1], F32, tag="va")
                nc.scalar.dma_start(qt_sb[:], QT[ib])
                nc.scalar.dma_start(kt_sb[:], KT[ib])
                nc.scalar.dma_start(
                    va_sb[:], VA[ib].rearrange("(c p) e -> p c e", p=P)
                )
                at_view = AT[ib].rearrange("(c p) t -> p c t", p=P)

                for ih in range(n_half):
                    s0 = ih * s_tile
                    ebig = ebig_pool.tile([P, n_chunk, s_tile], F32, tag="ebig")
                    z_ps = pz.tile([d + 1, s_tile], F32, tag="z")

                    for c in range(n_chunk):
                        st = pst.tile([P, s_tile], F32, tag="st")
                        for j in range(n_w):
                            nc.tensor.matmul(
                                st[:, j * w : (j + 1) * w],
                                lhsT=kt_sb[:, c * P : (c + 1) * P],
                                rhs=qt_sb[:, s0 + j * w : s0 + (j + 1) * w],
                                start=True,
                                stop=True,
                            )
                        nc.scalar.activation(
                            ebig[:, c, :], st[:, :], EXP,
                            bias=bias_sb[:, :], scale=scale,
                        )
                        # same-block (diagonal) part of the mask
                        ds0 = c * P
                        if s0 <= ds0 < s0 + s_tile:
                            off = ds0 - s0
                            nc.gpsimd.memset(
                                ebig[0:BLOCK, c, off : off + BLOCK], 0.0
                            )
                            nc.gpsimd.memset(
                                ebig[BLOCK:P, c, off + BLOCK : off + 2 * BLOCK], 0.0
                            )
                        for j in range(n_w):
                            nc.tensor.matmul(
                                z_ps[:, j * w : (j + 1) * w],
                                lhsT=va_sb[:, c, :],
                                rhs=ebig[:, c, j * w : (j + 1) * w],
                                start=(c == 0),
                                stop=(c == n_chunk - 1),
                            )

                    # epilogue: normalizer rbc = exp(-ln(sum)) broadcast to 128 rows
                    z_sb = zs_pool.tile([d + 1, s_tile], F32, tag="z_sb")
                    nc.vector.tensor_copy(z_sb[:, :], z_ps[:, :])
                    lnsum = sm1.tile([1, s_tile], F32, tag="lnsum")
                    nc.scalar.activation(lnsum[:, :], z_sb[d : d + 1, :], LOG)
                    r_ps = pr.tile([P, s_tile], F32, tag="r")
                    for j in range(n_w):
                        nc.tensor.matmul(
                            r_ps[:, j * w : (j + 1) * w],
                            lhsT=ones_sb[:, :],
                            rhs=lnsum[:, j * w : (j + 1) * w],
                            start=True,
                            stop=True,
                        )
                    rbc = sm1.tile([P, s_tile], F32, tag="rbc")
                    nc.scalar.activation(rbc[:, :], r_ps[:, :], EXP, scale=-1.0)

                    # normalize attn tiles in place (DVE, some chunks on GPSIMD)
                    for c in range(n_chunk):
                        eng = (
                            nc.gpsimd
                            if (c % gp_every == gp_every - 1)
                            else nc.vector
                        )
                        eng.tensor_mul(ebig[:, c, :], ebig[:, c, :], rbc[:, :])

                    # normalized out^T tile
                    ot = ot_pool.tile([d, s_tile], F32, tag="ot")
                    nc.gpsimd.tensor_mul(ot[:, :], z_sb[0:d, :], rbc[0:d, :])
                    nc.sync.dma_start(OT[ib][:, s0 : s0 + s_tile], ot[:, :])

                    # attn stores, 4 chunks per DMA
                    grp = 4 if n_chunk % 4 == 0 else 1
                    for g in range(n_chunk // grp):
                        nc.sync.dma_start(
                            at_view[:, g * grp : (g + 1) * grp, s0 : s0 + s_tile],
                            ebig[:, g * grp : (g + 1) * grp, :],
                        )

    nc.compile()
    return nc


_CACHE = {}
LAST_RESULTS = None  # BassKernelResults of the most recent kernel() call


def _get_nc():
    if "nc" not in _CACHE:
        _CACHE["nc"] = build_nc()
    return _CACHE["nc"]


def _make_mask():
    idx = np.arange(S)
    blk = idx // BLOCK
    return (blk[:, None] == blk[None, :]) | ((idx % BLOCK) == 0)[None, :]


def kernel(Q, K, V):
    global LAST_RESULTS
    Q = np.asarray(Q, dtype=np.float32).reshape(BH, S, D)
    K = np.asarray(K, dtype=np.float32).reshape(BH, S, D)
    V = np.asarray(V, dtype=np.float32).reshape(BH, S, D)

    bias = np.zeros((P, 1), np.float32)
    bias[0, 0] = NEG
    bias[BLOCK, 0] = NEG

    nc = _get_nc()
    in_maps = []
    for c in range(N_CORES):
        sl = slice(BH_PER_CORE * c, BH_PER_CORE * (c + 1))
        qt = np.ascontiguousarray(Q[sl].transpose(0, 2, 1))
        kt = np.ascontiguousarray(K[sl].transpose(0, 2, 1))
        va = np.ascontiguousarray(
            np.concatenate(
                [V[sl], np.ones((BH_PER_CORE, S, 1), np.float32)], axis=2
            )
        )
        in_maps.append({"qt": qt, "kt": kt, "va": va, "bias": bias})

    LAST_RESULTS = run_bass_kernel_spmd(
        nc, in_maps, core_ids=list(range(N_CORES))
    )

    attn = np.empty((BH, S, S), np.float32)
    out = np.empty((BH, S, D), np.float32)
    for c in range(N_CORES):
        r = LAST_RESULTS.results[c]
        sl = slice(BH_PER_CORE * c, BH_PER_CORE * (c + 1))
        attn[sl] = r["attnT"].transpose(0, 2, 1)
        out[sl] = r["outT"].transpose(0, 2, 1)

    return (
        out.reshape(B, H, S, D),
        attn.reshape(B, H, S, S),
        _make_mask(),
    )
